# revision 1
# baseline (speedup 1.0000x reference)
"""TRN2 Bass kernel for nn_GTLayer (ELL sparse attention, N=50000, K=16).

Sharding: nodes split contiguously across 8 NeuronCores (6250/core, padded
to 6272). Per core: embedding-gather -> h, PE matmuls -> q/k/v (k|v rows
interleaved), on-device AllGather of kv, then per 128-node tile 16
indirect-DMA neighbor-row gathers + DVE attention. Masking uses
t=(s+120)*mask, exp(0.25*t-30): masked lanes get exp(-30)~=1e-13, and a
fully-masked row degrades to the uniform average exactly like jax softmax.
"""
import numpy as np

import concourse.bass as bass
import concourse.mybir as mybir
import concourse.tile as tile
from concourse.masks import make_identity
from concourse.vector_clock import ScopedClock

F32 = mybir.dt.float32
I32 = mybir.dt.int32
U8 = mybir.dt.uint8
F16 = mybir.dt.float16
AX = mybir.AxisListType
ALU = mybir.AluOpType
AF = mybir.ActivationFunctionType

N_FEATS, VOCAB, HID, NH, HD, K = 9, 119, 128, 8, 16, 16
VFLAT = N_FEATS * VOCAB
P = 128
NCORES = 8
NRC = 6250          # real nodes per core
NPC = 6272          # padded nodes per core (49 x 128)

# ---------------------------------------------------------------- walrus fixes
# This walrus build rejects >1 sync-wait command per instruction. Two fixes:
# (1) TileContext tail drain: emit waits as single-wait nops.
# (2) General: split multi-wait instructions in the serialized BIR JSON by
#     inserting single-wait NoOps immediately before them (order preserved).


def _patched_drain_and_barrier(self, tick_clock, wait_clock):
    nc = self.nc
    probe = nc.sync.nop(nofuse=True)
    wait_clock.add_sem_waits(probe.ins, ScopedClock({None: tick_clock.global_clock}))
    waits = list(probe.ins.sync_info.on_wait or []) if probe.ins.sync_info else []
    if probe.ins.sync_info:
        probe.ins.sync_info.on_wait = waits[:1]
    for w in waits[1:]:
        n2 = nc.sync.nop(nofuse=True)
        if n2.ins.sync_info is None:
            n2.ins.sync_info = mybir.SyncInfo(on_update=[], on_wait=[w])
        else:
            n2.ins.sync_info.on_wait = [w]
    nc.sync.drain()
    nc.all_engine_barrier()
    assert self.sems is not None
    popped = nc._tile_sem_poison_stack.pop()
    assert popped is self._sem_poison
    nc.clear_and_free_semaphores(list(self.sems.allocated().values()))
    nc.all_engine_barrier()


tile.TileContext._drain_and_barrier = _patched_drain_and_barrier


def _split_waits_json(bir_bytes):
    import orjson
    m = orjson.loads(bir_bytes)
    n = 0
    for fn in m["functions"]:
        for blk in fn["blocks"]:
            new = []
            for ins in blk["instructions"]:
                si = ins.get("sync_info")
                waits = (si or {}).get("on_wait") or []
                if len(waits) > 1:
                    for w in waits[:-1]:
                        n += 1
                        new.append({
                            "debug": ins.get("debug", 0),
                            "engine": ins["engine"],
                            "ins": [], "name": f"I-wfix-{n}",
                            "opcode": "NoOp", "outs": [],
                            "sync_info": {"on_update": [], "on_wait": [w]},
                        })
                    si["on_wait"] = waits[-1:]
                new.append(ins)
            blk["instructions"] = new
    return orjson.dumps(m), n


import concourse.bass2jax as _b2j

_orig_cbk = _b2j.compile_bir_kernel


def _patched_cbk(ant_bir_str, *a, **kw):
    fixed, n = _split_waits_json(ant_bir_str)
    return _orig_cbk(fixed, *a, **kw)


_b2j.compile_bir_kernel = _patched_cbk

# ---------------------------------------------------------------- device code


def build(nc, npad_core=NPC, ncores=NCORES):
    T = npad_core // P
    ntot = npad_core * ncores

    xc = nc.dram_tensor("xc", [npad_core, N_FEATS], I32, kind="ExternalInput")
    nb = nc.dram_tensor("nb", [npad_core, K], I32, kind="ExternalInput")
    mk = nc.dram_tensor("mk", [npad_core, K], U8, kind="ExternalInput")
    emb = nc.dram_tensor("emb", [VFLAT, HID], F32, kind="ExternalInput")
    wq = nc.dram_tensor("wq", [HID, HID], F32, kind="ExternalInput")
    wk = nc.dram_tensor("wk", [HID, HID], F32, kind="ExternalInput")
    wv = nc.dram_tensor("wv", [HID, HID], F32, kind="ExternalInput")
    bq = nc.dram_tensor("bq", [HID, 1], F32, kind="ExternalInput")
    bk = nc.dram_tensor("bk", [HID, 1], F32, kind="ExternalInput")
    bv = nc.dram_tensor("bv", [HID, 1], F32, kind="ExternalInput")
    out = nc.dram_tensor("out", [npad_core, HID], F32, kind="ExternalOutput")

    with tile.TileContext(nc) as tc:
        with (
            tc.tile_pool(name="const", bufs=1) as cp,
            tc.tile_pool(name="resident", bufs=1) as rp,
            tc.tile_pool(name="work", bufs=3) as wp,
            tc.tile_pool(name="gath", bufs=3) as gp,
            tc.tile_pool(name="psum", bufs=2, space="PSUM") as pp,
            tc.tile_pool(name="dram", bufs=1, space="DRAM") as dp,
        ):
            ident = cp.tile([P, P], F32, name="ident")
            make_identity(nc, ident[:])
            negc = cp.tile([P, 1], F32, name="negc")
            nc.gpsimd.memset(negc[:], -30.0)
            w_q = cp.tile([HID, HID], F32, name="w_q")
            w_k = cp.tile([HID, HID], F32, name="w_k")
            w_v = cp.tile([HID, HID], F32, name="w_v")
            b_q = cp.tile([HID, 1], F32, name="b_q")
            b_k = cp.tile([HID, 1], F32, name="b_k")
            b_v = cp.tile([HID, 1], F32, name="b_v")
            for t_, d_ in ((w_q, wq), (w_k, wk), (w_v, wv),
                           (b_q, bq), (b_k, bk), (b_v, bv)):
                nc.sync.dma_start(out=t_[:], in_=d_[:])

            q_all = rp.tile([P, T * HID], F16, name="q_all")
            idx_all = rp.tile([P, T * K], I32, name="idx_all")
            msk_all = rp.tile([P, T * K], F32, name="msk_all")

            kv_shard = dp.tile([npad_core, 2 * HID], F16, name="kv_shard")
            kv_full = dp.tile([ntot, 2 * HID], F16, name="kv_full",
                              addr_space="Shared")

            # phase 1: h -> q,k,v
            for t in range(T):
                r0 = t * P
                xt = wp.tile([P, N_FEATS], I32, name="xt")
                nc.sync.dma_start(out=xt[:], in_=xc[r0:r0 + P, :])
                mt8 = wp.tile([P, K], U8, name="mt8")
                nc.sync.dma_start(out=mt8[:], in_=mk[r0:r0 + P, :])
                nc.vector.tensor_copy(out=msk_all[:, t * K:(t + 1) * K], in_=mt8[:])
                nc.sync.dma_start(out=idx_all[:, t * K:(t + 1) * K],
                                  in_=nb[r0:r0 + P, :])

                et = wp.tile([P, N_FEATS * HID], F32, name="et")
                for f in range(N_FEATS):
                    nc.gpsimd.indirect_dma_start(
                        out=et[:, f * HID:(f + 1) * HID], out_offset=None,
                        in_=emb[:],
                        in_offset=bass.IndirectOffsetOnAxis(
                            ap=xt[:, f:f + 1], axis=0))
                ht = wp.tile([P, HID], F32, name="ht")
                nc.vector.tensor_reduce(
                    out=ht[:],
                    in_=et[:].rearrange("p (f c) -> p c f", f=N_FEATS),
                    axis=AX.X, op=ALU.add)

                hT_p = pp.tile([P, P], F32, name="hT_p", space="PSUM")
                nc.tensor.transpose(out=hT_p[:], in_=ht[:], identity=ident[:])
                hT = wp.tile([P, P], F32, name="hT")
                nc.scalar.copy(out=hT[:], in_=hT_p[:])

                for wmat, bias, dst in (
                        (w_q, b_q, "q"), (w_k, b_k, "k"), (w_v, b_v, "v")):
                    yT_p = pp.tile([P, P], F32, name="yT_p", space="PSUM")
                    nc.tensor.matmul(out=yT_p[:], lhsT=wmat[:], rhs=hT[:],
                                     start=True, stop=True)
                    yT = wp.tile([P, P], F32, name="yT")
                    nc.vector.tensor_scalar_add(out=yT[:], in0=yT_p[:],
                                                scalar1=bias[:])
                    y_p = pp.tile([P, P], F32, name="y_p", space="PSUM")
                    nc.tensor.transpose(out=y_p[:], in_=yT[:], identity=ident[:])
                    if dst == "q":
                        nc.scalar.copy(out=q_all[:, t * HID:(t + 1) * HID],
                                       in_=y_p[:])
                    elif dst == "k":
                        kvt = wp.tile([P, 2 * HID], F16, name="kvt")
                        nc.scalar.copy(out=kvt[:, 0:HID], in_=y_p[:])
                    else:
                        nc.scalar.copy(out=kvt[:, HID:2 * HID], in_=y_p[:])
                        nc.sync.dma_start(out=kv_shard[r0:r0 + P, :], in_=kvt[:])

            # phase 2: allgather kv across the 8 cores
            nc.gpsimd.collective_compute(
                "AllGather", ALU.bypass,
                replica_groups=[list(range(ncores))],
                ins=[kv_shard[:]], outs=[kv_full[:]])

            # phase 3: neighbor gather + attention
            lp = nc.allow_low_precision(reason="fp16 attention scores")
            lp.__enter__()
            for t in range(T):
                r0 = t * P
                knvn = gp.tile([P, K * 2 * HID], F16, name="knvn")
                for j in range(K):
                    nc.gpsimd.indirect_dma_start(
                        out=knvn[:, j * 2 * HID:(j + 1) * 2 * HID],
                        out_offset=None, in_=kv_full[:],
                        in_offset=bass.IndirectOffsetOnAxis(
                            ap=idx_all[:, t * K + j:t * K + j + 1], axis=0))
                kn = knvn[:].rearrange("p (j c) -> p j c", j=K)[:, :, 0:HID]
                vn = knvn[:].rearrange("p (j c) -> p j c", j=K)[:, :, HID:2 * HID]

                qb = q_all[:, t * HID:(t + 1) * HID] \
                    .rearrange("p (a c) -> p a c", a=1).to_broadcast([P, K, HID])
                prod = wp.tile([P, K * HID], F32, name="prod")
                nc.vector.tensor_tensor(
                    out=prod[:].rearrange("p (j c) -> p j c", j=K),
                    in0=kn, in1=qb, op=ALU.mult)

                s = wp.tile([P, K * NH], F32, name="s")
                nc.vector.tensor_reduce(
                    out=s[:],
                    in_=prod[:].rearrange("p (j h d) -> p j h d", j=K, h=NH),
                    axis=AX.X, op=ALU.add)

                mb = msk_all[:, t * K:(t + 1) * K] \
                    .rearrange("p (j a) -> p j a", a=1).to_broadcast([P, K, NH])
                tt = wp.tile([P, K * NH], F32, name="tt")
                nc.vector.scalar_tensor_tensor(
                    out=tt[:].rearrange("p (j h) -> p j h", j=K),
                    in0=s[:].rearrange("p (j h) -> p j h", j=K),
                    scalar=120.0, in1=mb, op0=ALU.add, op1=ALU.mult)

                e = wp.tile([P, K * NH], F32, name="e")
                nc.scalar.activation(out=e[:], in_=tt[:], func=AF.Exp,
                                     bias=negc[:], scale=0.25)

                z = wp.tile([P, NH], F32, name="z")
                nc.vector.tensor_reduce(
                    out=z[:], in_=e[:].rearrange("p (j h) -> p h j", j=K),
                    axis=AX.X, op=ALU.add)
                zr = wp.tile([P, NH], F32, name="zr")
                nc.vector.reciprocal(out=zr[:], in_=z[:])


                at = wp.tile([P, K * NH], F32, name="at")
                nc.vector.tensor_tensor(
                    out=at[:].rearrange("p (j h) -> p j h", j=K),
                    in0=e[:].rearrange("p (j h) -> p j h", j=K),
                    in1=zr[:].rearrange("p (a h) -> p a h", a=1)
                        .to_broadcast([P, K, NH]),
                    op=ALU.mult)

                prod2 = wp.tile([P, K * HID], F32, name="prod2")
                nc.vector.tensor_tensor(
                    out=prod2[:].rearrange("p (j h d) -> p j h d", j=K, h=NH),
                    in0=vn.rearrange("p j (h d) -> p j h d", h=NH),
                    in1=at[:].rearrange("p (j h) -> p j h", j=K)
                        .rearrange("p j (h a) -> p j h a", a=1)
                        .to_broadcast([P, K, NH, HD]),
                    op=ALU.mult)

                o = wp.tile([P, HID], F32, name="o")
                nc.vector.tensor_reduce(
                    out=o[:],
                    in_=prod2[:].rearrange("p (j c) -> p c j", j=K),
                    axis=AX.X, op=ALU.add)
                nc.sync.dma_start(out=out[r0:r0 + P, :], in_=o[:])
            lp.__exit__(None, None, None)
    return nc


# ---------------------------------------------------------------- host side


def _prep(X, nbr_idx, nbr_mask, atom_emb, Wq, bq, Wk, bk, Wv, bv):
    offs = (np.arange(N_FEATS, dtype=np.int64) * VOCAB)[None, :]
    xc_full = (np.asarray(X).astype(np.int64) + offs).astype(np.int32)
    g = np.asarray(nbr_idx).astype(np.int64)
    remap = ((g // NRC) * NPC + (g % NRC)).astype(np.int32)
    mask = np.asarray(nbr_mask).astype(np.uint8)
    emb_flat = np.ascontiguousarray(
        np.asarray(atom_emb, dtype=np.float32).reshape(VFLAT, HID))
    maps = []
    for r in range(NCORES):
        lo, hi = r * NRC, (r + 1) * NRC
        xcp = np.zeros((NPC, N_FEATS), np.int32)
        xcp[:NRC] = xc_full[lo:hi]
        nbp = np.zeros((NPC, K), np.int32)
        nbp[:NRC] = remap[lo:hi]
        mkp = np.zeros((NPC, K), np.uint8)
        mkp[:NRC] = mask[lo:hi]
        maps.append({
            "xc": xcp, "nb": nbp, "mk": mkp, "emb": emb_flat,
            "wq": np.ascontiguousarray(np.asarray(Wq, np.float32)),
            "wk": np.ascontiguousarray(np.asarray(Wk, np.float32)),
            "wv": np.ascontiguousarray(np.asarray(Wv, np.float32)),
            "bq": np.asarray(bq, np.float32).reshape(HID, 1),
            "bk": np.asarray(bk, np.float32).reshape(HID, 1),
            "bv": np.asarray(bv, np.float32).reshape(HID, 1),
        })
    return maps


_CACHE = {}


def run_on_device(maps, trace=False):
    from concourse.bass_utils import run_bass_kernel_spmd
    if "nc" not in _CACHE:
        nc = bass.Bass()
        build(nc)
        _CACHE["nc"] = nc
    return run_bass_kernel_spmd(_CACHE["nc"], maps, list(range(NCORES)),
                                trace=trace)


def kernel(X, nbr_idx, nbr_mask, atom_emb, Wq, bq, Wk, bk, Wv, bv):
    maps = _prep(X, nbr_idx, nbr_mask, atom_emb, Wq, bq, Wk, bk, Wv, bv)
    res = run_on_device(maps)
    return np.concatenate([r["out"][:NRC] for r in res.results], axis=0)

